# revision 3
# baseline (speedup 1.0000x reference)
"""GCN block (edge-dropout GCN conv + BatchNorm + node dropout) on 8 Trainium2
NeuronCores — v3.

On top of v2 (scan-based degrees, 256B hn rows, batched one-hot builds):
  - hn rows are PERMUTED: batch b of par p lands in block 2b+p, partition-major
    within the block (row = (2b+p)*896 + p_idx*7 + t).  Each hn write is then
    128 descriptors of 1792B (no sub-512B RMW penalty), and rows written by
    both pair members form a growing prefix.
  - TWO pair barriers: after half the hn writes (rows [0,25088) complete) and
    after all of them.  Gathers are split into three row ranges
    r0=[0,25088) / r1=[25088,32768) / r2=[32768,50176); r0 gathers (about half
    the edges) overlap the second half of phase C.
  - Window aggregation runs in two passes: pass 1 accumulates r0 chunks in
    PSUM and spills bf16 partials to SBUF; pass 2 accumulates r1/r2 chunks
    and re-adds the spilled partial with an identity matmul (no extra
    DVE/ACT work on the critical path).
  - Finale scale/shift math in bf16 (2x DVE rate).
"""

import sys

import numpy as np

for _p in ("/opt/trn_rl_repo", "/opt/pypackages"):
    if _p not in sys.path:
        sys.path.append(_p)

import concourse.bacc as bacc
import concourse.bass as bass
import concourse.mybir as mybir
import concourse.tile as tile
from concourse import library_config
from concourse.bass import _add_dep_helper
from concourse.bass_utils import run_bass_kernel_spmd

F32 = mybir.dt.float32
BF16 = mybir.dt.bfloat16
I16 = mybir.dt.int16
AF = mybir.ActivationFunctionType
OP = mybir.AluOpType

N_NODES = 50000
IN_FEAT = 256
OUT_FEAT = 128
P_EDGE = 0.2
P_NODE = 0.1
BN_EPS = 1e-5
CORES = 8
NPAD = 50176  # 8 * 49 * 128
CHALF = NPAD // 2  # 25088, rows complete at barrier 1
LO_ROWS = 32768  # int16 index limit for the low gather range
OHB = 8  # one-hot chunks built per DVE instruction
NB = 7   # node tiles per phase-C batch (block = NB*128 = 896 rows)
R_BASE = (0, CHALF, LO_ROWS)
R_END = (CHALF, LO_ROWS, NPAD)


def _r128(x):
    return (int(x) + 127) // 128 * 128


def _r16(x):
    return (int(x) + 15) // 16 * 16


def _wrap16(flat, reps=8):
    """[L] -> [16*reps, L//16]: element j at row j%16 (replicated), col j//16."""
    a = flat.reshape(-1, 16).T  # [16, L//16]
    return np.tile(a, (reps, 1))


def _perm_rows(node):
    """hn row for padded node id: batch-interleaved, partition-major."""
    node = np.asarray(node)
    p = node // CHALF
    loc = node % CHALF
    b = loc // (NB * 128)
    t = (loc % (NB * 128)) // 128
    pp = loc % 128
    return (2 * b + p) * (NB * 128) + pp * NB + t


def _slot_table(local_idx, er_vals, npc, K):
    """Fixed-slot scan table: row r=local//64 holds nodes r*64..r*64+63, node
    n's edges at cols [1 + (n%64)*K, ...). Returns [npc//64, 64*K+1] f32."""
    nrows = npc // 64
    W = 64 * K + 1
    tab = np.zeros((nrows, W), np.float32)
    if len(local_idx) == 0:
        return tab
    order = np.argsort(local_idx, kind="stable")
    li = local_idx[order]
    ev = er_vals[order]
    cnt = np.bincount(li, minlength=npc)
    assert cnt.max() <= K, (cnt.max(), K)
    start = np.concatenate([[0], np.cumsum(cnt)])
    pos = np.arange(len(li)) - start[li]
    row = li // 64
    col = 1 + (li % 64) * K + pos
    tab[row, col] = ev
    return tab


def prep_inputs(features, W, gamma, beta, src, dst, edge_rand, node_rand,
                n_nodes=N_NODES, npad=NPAD):
    """Host-side sharding/layout. Returns (shapes, per_core_input_maps)."""
    cores = CORES
    npc = npad // cores
    nw = npc // 128
    nw2 = nw * 2
    fin = features.shape[1]

    src = np.asarray(src).astype(np.int64)
    dst = np.asarray(dst).astype(np.int64)
    er = np.asarray(edge_rand).astype(np.float32)

    # global slot sizes (uniform across cores so SPMD shapes match)
    Kd = _r16(max(1, int(np.bincount(dst, minlength=n_nodes).max())))
    Ks = _r16(max(1, int(np.bincount(src, minlength=n_nodes).max())))
    Wd = 64 * Kd + 1
    Ws = 64 * Ks + 1

    # ---------- dst shard: (owner core, window, src row range) ----------
    d_owner = dst // npc
    nseg = nw * 3
    src_row = _perm_rows(src)
    src_r = np.where(src_row < CHALF, 0, np.where(src_row < LO_ROWS, 1, 2))

    per_core = []
    cnt = np.zeros((cores, nw, 3), np.int64)
    for c in range(cores):
        m = d_owner == c
        s_c, d_c, e_c, r_c = src_row[m], dst[m], er[m], src_r[m]
        key = (d_c % npc) // 128 * 3 + r_c
        o = np.argsort(key, kind="stable")
        s_c, d_c, e_c, key = s_c[o], d_c[o], e_c[o], key[o]
        cc = np.bincount(key, minlength=nseg)
        cnt[c] = cc.reshape(nw, 3)
        per_core.append((s_c, d_c, e_c, key, cc))

    caps = np.zeros((nw, 3), np.int64)
    for w in range(nw):
        for r in range(3):
            mx = cnt[:, w, r].max()
            caps[w, r] = _r128(mx) if mx > 0 else 0
    # group-major global layout: per group of GWIN windows, r0 segments of all
    # windows, then r1, then r2.  seg id = w*3 + r.
    GWIN = 2
    groups_w = [list(range(g, min(g + GWIN, nw)))
                for g in range(0, nw, GWIN)]
    seg_order = []
    for ws in groups_w:
        for r in range(3):
            for w in ws:
                seg_order.append(w * 3 + r)
    off = 0
    seg_off_map = np.zeros(nseg, np.int64)
    for sid in seg_order:
        seg_off_map[sid] = off
        off += caps.reshape(-1)[sid]
    totcap = int(off)
    nch_d = totcap // 128

    # ---------- shared constant inputs ----------
    bf16 = np.dtype("bfloat16")
    featT_full = np.zeros((fin, npad), np.float32)
    featT_full[:, :n_nodes] = np.asarray(features).astype(np.float32).T
    featT_halves = [featT_full[:, :CHALF].astype(bf16),
                    featT_full[:, CHALF:].astype(bf16)]
    io8 = np.tile(np.arange(128, dtype=np.float32), OHB)[None, :].repeat(
        128, axis=0).astype(bf16)
    ident = np.eye(128, dtype=np.float32)
    ident16 = np.eye(128, dtype=np.float32).astype(bf16)
    ones_row = np.ones((1, 128), np.float32)
    gam = np.asarray(gamma).astype(np.float32).reshape(1, OUT_FEAT)
    bet = np.asarray(beta).astype(np.float32).reshape(1, OUT_FEAT)
    nrand = np.asarray(node_rand).astype(np.float32)
    w_bf = np.asarray(W).astype(np.float32).astype(bf16)

    s_owner = src // npc

    in_maps = []
    for c in range(cores):
        s_c, d_c, e_c, key, cc = per_core[c]
        data_off = np.concatenate([[0], np.cumsum(cc)])
        pos_in_seg = np.arange(len(s_c)) - data_off[key]
        tgt = seg_off_map[key] + pos_in_seg

        # pad slots hold a VALID index (row 0 of the range) so every gather
        # writes its full capacity; dstl=-1 kills their contribution.
        idxf = np.zeros(max(totcap, 1), np.int64)
        dstlf = np.full(max(nch_d * 128, 1), -1.0, np.float32)
        erf = np.zeros(max(nch_d * 128, 1), np.float32)
        w_of = (d_c % npc) // 128
        r_of = key % 3
        lidx = s_c - np.asarray(R_BASE)[r_of]
        idxf[tgt] = lidx
        dstlf[tgt] = (d_c % npc) - w_of * 128
        erf[tgt] = e_c
        if len(lidx):
            assert int(lidx.max()) < 32768 and int(lidx.min()) >= 0
        idx16 = _wrap16(idxf.astype(np.int16))
        dstl_t = np.ascontiguousarray(dstlf.reshape(-1, 128).T)
        er_t = np.ascontiguousarray(erf.reshape(-1, 128).T)

        # deg_dst scan table: this core's dst edges, slot-placed by dst-local
        dtab = _slot_table((d_c % npc).astype(np.int64), e_c, npc, Kd)

        # deg_src scan table: edges whose src is owned by this core
        ms = s_owner == c
        stab = _slot_table((src[ms] % npc).astype(np.int64), er[ms], npc, Ks)

        nr = np.ones((npc, OUT_FEAT), np.float32)
        lo_n = c * npc
        hi_n = min((c + 1) * npc, n_nodes)
        if hi_n > lo_n:
            nr[: hi_n - lo_n] = nrand[lo_n:hi_n]

        in_maps.append({
            "featT": featT_halves[c % 2],
            "w_mat": w_bf,
            "gam": gam, "bet": bet, "io8": io8,
            "ident": ident, "ident16": ident16, "ones_row": ones_row,
            "idx16": idx16, "dstl": dstl_t, "erd": er_t,
            "dtab": dtab, "stab": stab,
            "noder": nr,
        })

    shapes = dict(npad=npad, npc=npc, nw=nw, nw2=nw2, fin=fin,
                  nch_d=max(nch_d, 1),
                  totcap=max(totcap, 1), Kd=Kd, Ks=Ks, Wd=Wd, Ws=Ws,
                  caps=caps, seg_off_map=seg_off_map,
                  groups_w=groups_w, n_nodes=n_nodes)
    return shapes, in_maps


def build_program(sh, nocc=False, **_ignored):
    npad, npc, nw, nw2, fin = (sh["npad"], sh["npc"], sh["nw"], sh["nw2"],
                               sh["fin"])
    caps = sh["caps"]
    seg_off_map = sh["seg_off_map"]
    groups_w = sh["groups_w"]
    n_nodes = sh["n_nodes"]
    Kd, Ks, Wd, Ws = sh["Kd"], sh["Ks"], sh["Wd"], sh["Ws"]
    nt = npad // 128          # node tiles
    kt = fin // 128           # contraction tiles for features @ W

    nc = bacc.Bacc("TRN2", target_bir_lowering=False, debug=False,
                   num_devices=CORES, num_swdge_queues=4)

    featT = nc.dram_tensor("featT", [fin, CHALF], BF16, kind="ExternalInput")
    w_mat = nc.dram_tensor("w_mat", [fin, OUT_FEAT], BF16, kind="ExternalInput")
    gam = nc.dram_tensor("gam", [1, OUT_FEAT], F32, kind="ExternalInput")
    bet = nc.dram_tensor("bet", [1, OUT_FEAT], F32, kind="ExternalInput")
    io8_d = nc.dram_tensor("io8", [128, OHB * 128], BF16, kind="ExternalInput")
    ident = nc.dram_tensor("ident", [128, 128], F32, kind="ExternalInput")
    ident16 = nc.dram_tensor("ident16", [128, 128], BF16, kind="ExternalInput")
    ones_row = nc.dram_tensor("ones_row", [1, 128], F32, kind="ExternalInput")
    idx16 = nc.dram_tensor("idx16", [128, sh["totcap"] // 16], I16,
                           kind="ExternalInput")
    dstl = nc.dram_tensor("dstl", [128, sh["nch_d"]], F32, kind="ExternalInput")
    erd = nc.dram_tensor("erd", [128, sh["nch_d"]], F32, kind="ExternalInput")
    dtab = nc.dram_tensor("dtab", [nw2, Wd], F32, kind="ExternalInput")
    stab = nc.dram_tensor("stab", [nw2, Ws], F32, kind="ExternalInput")
    noder = nc.dram_tensor("noder", [npc, OUT_FEAT], F32, kind="ExternalInput")
    out = nc.dram_tensor("out", [npc, OUT_FEAT], F32, kind="ExternalOutput")

    hn = nc.dram_tensor("hn", [npad, OUT_FEAT], BF16, addr_space="Shared")
    # read-alias of hn at the same address: the gathers read through this
    # handle so the dep tracker doesn't serialize them behind ALL hn writes
    # (the write->read ordering is enforced by the pair barriers, which is
    # the cross-core requirement anyway).
    hn_r = nc.dram_tensor("hn_r", [npad, OUT_FEAT], BF16, addr_space="Shared")
    nc.lookup_mloc(hn_r).addr = nc.lookup_mloc(hn).addr
    barr1_in = nc.dram_tensor("barr1_in", [1, 128], F32)
    barr1_out = nc.dram_tensor("barr1_out", [1, 128], F32)
    barr_in = nc.dram_tensor("barr_in", [1, 128], F32)
    barr_out = nc.dram_tensor("barr_out", [1, 128], F32)
    degb_in = nc.dram_tensor("degb_in", [1, npc], F32)
    degb_out = nc.dram_tensor("degb_out", [nt, 128], F32)
    statb_in = nc.dram_tensor("statb_in", [1, 2 * OUT_FEAT], F32)
    statb_out = nc.dram_tensor("statb_out", [1, 2 * OUT_FEAT], F32)

    groups = [list(range(CORES))]
    pair_groups = [[2 * i, 2 * i + 1] for i in range(CORES // 2)]

    with tile.TileContext(nc) as tc:
        nc.gpsimd.load_library(library_config.mlp)
        with (
            tc.tile_pool(name="const", bufs=1) as cpool,
            tc.tile_pool(name="aux", bufs=1) as apool,
            tc.tile_pool(name="work", bufs=1) as wpool,
            tc.tile_pool(name="psum", bufs=1, space="PSUM") as pps,
        ):
            # ---------- constants ----------
            w_tiles = []
            for k in range(kt):
                wt = cpool.tile([128, OUT_FEAT], BF16, tag=f"wk{k}", name=f"wk{k}")
                nc.sync.dma_start(out=wt[:, :], in_=w_mat[k * 128:(k + 1) * 128, :])
                w_tiles.append(wt)
            io8_sb = cpool.tile([128, OHB * 128], BF16, tag="io8", name="io8")
            nc.sync.dma_start(out=io8_sb[:, :], in_=io8_d[:, :])
            idn = cpool.tile([128, 128], F32, tag="idn", name="idn")
            nc.sync.dma_start(out=idn[:, :], in_=ident[:, :])
            idn16 = cpool.tile([128, 128], BF16, tag="idn16", name="idn16")
            nc.sync.dma_start(out=idn16[:, :], in_=ident16[:, :])
            onesr = cpool.tile([1, 128], F32, tag="onesr", name="onesr")
            nc.sync.dma_start(out=onesr[:, :], in_=ones_row[:, :])
            ones16 = cpool.tile([128, 1], BF16, tag="ones16", name="ones16")
            nc.vector.memset(ones16[:, :], 1.0)
            gam_sb = cpool.tile([1, OUT_FEAT], F32, tag="gam_sb", name="gam_sb")
            nc.sync.dma_start(out=gam_sb[:, :], in_=gam[:, :])
            bet_sb = cpool.tile([1, OUT_FEAT], F32, tag="bet_sb", name="bet_sb")
            nc.sync.dma_start(out=bet_sb[:, :], in_=bet[:, :])

            # ---------- phase B: degrees via fixed-slot scans ----------
            with tc.tile_pool(name="scan", bufs=1) as spool, \
                    tc.tile_pool(name="scan_ps", bufs=1, space="PSUM") as sps:
                # deg_src -> degb_in -> AllGather
                st_sb = spool.tile([nw2, Ws], F32, tag="st_sb", name="st_sb")
                nc.sync.dma_start(out=st_sb[:, :], in_=stab[:, :])
                sk = spool.tile([nw2, Ws], F32, tag="sk", name="sk")
                nc.vector.tensor_scalar(sk[:, :], st_sb[:, :], P_EDGE, None,
                                        op0=OP.is_ge)
                ssc = spool.tile([nw2, Ws], F32, tag="ssc", name="ssc")
                nc.vector.tensor_tensor_scan(ssc[:, :], sk[:, :], sk[:, :],
                                             0.0, op0=OP.add, op1=OP.bypass)
                degs = spool.tile([nw2, 64], F32, tag="degs", name="degs")
                nc.vector.tensor_sub(degs[:, :], ssc[:, Ks::Ks],
                                     ssc[:, 0:64 * Ks:Ks])
                nc.sync.dma_start(
                    out=degb_in.ap().rearrange("o (p j) -> p (o j)", p=nw2),
                    in_=degs[:, :])
                if nocc:
                    nc.sync.dma_start(
                        out=degb_out[0:npc // 128, :],
                        in_=degb_in.ap().rearrange("o (r c) -> (o r) c", c=128))
                else:
                    nc.gpsimd.collective_compute(
                        "AllGather", OP.bypass, replica_groups=groups,
                        ins=[degb_in.ap().opt()], outs=[degb_out.ap().opt()])

                # deg_dst (local, no collective) -> d3T [128, nw]
                dt_sb = spool.tile([nw2, Wd], F32, tag="dt_sb", name="dt_sb")
                nc.sync.dma_start(out=dt_sb[:, :], in_=dtab[:, :])
                dk = spool.tile([nw2, Wd], F32, tag="dk", name="dk")
                nc.vector.tensor_scalar(dk[:, :], dt_sb[:, :], P_EDGE, None,
                                        op0=OP.is_ge)
                dsc = spool.tile([nw2, Wd], F32, tag="dsc", name="dsc")
                nc.vector.tensor_tensor_scan(dsc[:, :], dk[:, :], dk[:, :],
                                             0.0, op0=OP.add, op1=OP.bypass)
                degd = spool.tile([nw2, 64], F32, tag="degd", name="degd")
                nc.vector.tensor_sub(degd[:, :], dsc[:, Kd::Kd],
                                     dsc[:, 0:64 * Kd:Kd])
                tp64 = sps.tile([64, nw2], F32, tag="tp64", name="tp64")
                nc.tensor.transpose(tp64[:, 0:nw2], degd[0:nw2, 0:64],
                                    idn[0:nw2, 0:nw2])
                d1 = wpool.tile([128, nw], F32, tag="d1", name="d1")
                nc.vector.tensor_scalar_max(d1[0:64, :], tp64[:, 0::2], 1.0)
                nc.vector.tensor_scalar_max(d1[64:128, :], tp64[:, 1::2], 1.0)
                d2 = wpool.tile([128, nw], F32, tag="d2", name="d2")
                nc.scalar.sqrt(d2[:, :], d1[:, :])
                d3T = wpool.tile([128, nw], F32, tag="d3T", name="d3T")
                nc.vector.reciprocal(d3T[:, :], d2[:, :])

            # reload MY half's degrees -> [128, nt//2] rsqrt(max(deg,1))
            nt_half = nt // 2
            par = nc.sync.partition_id() % 2
            rbase = par * nt_half
            rdegs = wpool.tile([128, nt_half], F32, tag="rdegs", name="rdegs")
            with tc.tile_pool(name="degld", bufs=2) as dl_pool, \
                    tc.tile_pool(name="degt_ps", bufs=2, space="PSUM") as tps_pool:
                for blk in range((nt_half + 127) // 128):
                    r0 = blk * 128
                    r1 = min(r0 + 128, nt_half)
                    nrow = r1 - r0
                    dl = dl_pool.tile([128, 128], F32, tag="dl", name=f"dl{blk}")
                    nc.sync.dma_start(
                        out=dl[0:nrow, :],
                        in_=degb_out[bass.ds(rbase + r0, nrow), :])
                    tp = tps_pool.tile([128, 128], F32, tag="tp", name=f"tp{blk}")
                    nc.tensor.transpose(tp[:, 0:nrow], dl[0:nrow, :],
                                        idn[0:nrow, 0:nrow])
                    t1 = dl_pool.tile([128, 128], F32, tag="t1", name=f"t1{blk}")
                    nc.vector.tensor_scalar_max(t1[:, 0:nrow], tp[:, 0:nrow], 1.0)
                    t2 = dl_pool.tile([128, 128], F32, tag="t2", name=f"t2{blk}")
                    nc.scalar.sqrt(t2[:, 0:nrow], t1[:, 0:nrow])
                    nc.vector.reciprocal(rdegs[:, r0:r1], t2[:, 0:nrow])

            # ---------- phase D prologue: edge tables (needed by the r0
            # gathers that overlap phase C, so emitted before it) ----------
            idx_sb = apool.tile([128, sh["totcap"] // 16], I16, tag="idx_sb",
                                name="idx_sb")
            nc.sync.dma_start(out=idx_sb[:, :], in_=idx16[:, :])
            dstl_sb = apool.tile([128, sh["nch_d"]], F32, tag="dstl_sb",
                                 name="dstl_sb")
            nc.sync.dma_start(out=dstl_sb[:, :], in_=dstl[:, :])
            erd_sb = apool.tile([128, sh["nch_d"]], F32, tag="erd_sb",
                                name="erd_sb")
            nc.sync.dma_start(out=erd_sb[:, :], in_=erd[:, :])
            # dm = keep*(dstl+1)-1 (bf16, exact small ints; -1 for dropped/pad)
            keep_d = apool.tile([128, sh["nch_d"]], F32, tag="keep_d",
                                name="keep_d")
            nc.vector.tensor_scalar(keep_d[:, :], erd_sb[:, :], P_EDGE, None,
                                    op0=OP.is_ge)
            dmf = apool.tile([128, sh["nch_d"]], F32, tag="dmf", name="dmf")
            nc.vector.tensor_scalar(dmf[:, :], dstl_sb[:, :], 1.0, None,
                                    op0=OP.add)
            nc.vector.tensor_mul(dmf[:, :], dmf[:, :], keep_d[:, :])
            dm = apool.tile([128, sh["nch_d"]], BF16, tag="dm", name="dm")
            nc.vector.tensor_scalar(dm[:, :], dmf[:, :], 1.0, None,
                                    op0=OP.subtract)

            # ---------- phase C: hn table, permuted blocks ----------
            assert nt_half % NB == 0
            nbatch = nt_half // NB  # 28
            half_b = nbatch // 2    # barrier 1 after this many batches
            hn_writes = []
            barrier1_cc = None
            max_g0 = max((sum(int(caps[w, 0]) for w in ws)
                          for ws in groups_w), default=128)
            max_g12 = max((sum(int(caps[w, 1] + caps[w, 2]) for w in ws)
                           for ws in groups_w), default=128)
            with tc.tile_pool(name="hload", bufs=6) as hl_pool, \
                    tc.tile_pool(name="hps", bufs=6, space="PSUM") as hps_pool, \
                    tc.tile_pool(name="hout", bufs=6) as ho_pool:
                for b in range(nbatch):
                    n0 = b * NB * 128
                    ft = hl_pool.tile([128, kt * NB * 128], BF16, tag="ft",
                                      name=f"ft{b}")
                    nc.scalar.dma_start(
                        out=ft[:, :].rearrange("p (k n) -> p k n", k=kt),
                        in_=featT[:, n0:n0 + NB * 128].rearrange(
                            "(k p) n -> p k n", k=kt))
                    hnt = ho_pool.tile([128, NB * OUT_FEAT], BF16, tag="hnt",
                                       name=f"hnt{b}")
                    for j in range(NB):
                        hps = hps_pool.tile([128, OUT_FEAT], F32, tag="hps",
                                            name=f"hps{b}_{j}")
                        for k in range(kt):
                            nc.tensor.matmul(
                                hps[:, :],
                                lhsT=ft[:, (k * NB + j) * 128:
                                        (k * NB + j + 1) * 128],
                                rhs=w_tiles[k][:, :],
                                start=(k == 0), stop=(k == kt - 1))
                        t = b * NB + j
                        h_sl = hnt[:, j * OUT_FEAT:(j + 1) * OUT_FEAT]
                        if j % 2 == 0:
                            nc.scalar.activation(h_sl, hps[:, :], AF.Copy,
                                                 scale=rdegs[:, t:t + 1])
                        else:
                            nc.vector.tensor_scalar(h_sl, hps[:, :],
                                                    rdegs[:, t:t + 1], None,
                                                    op0=OP.mult)
                    # block 2b+par, partition-major rows (desc = 128 x 1792B)
                    blk_row = (2 * b + par) * (NB * 128)
                    wr = nc.sync.dma_start(
                        out=hn[bass.ds(blk_row, NB * 128), :].rearrange(
                            "(p t) f -> p t f", p=128),
                        in_=hnt[:, :].rearrange("p (t f) -> p t f", t=NB))
                    hn_writes.append(wr)
                    if b == half_b - 1:
                        # barrier 1: rows [0, CHALF) written by both members
                        bw1 = nc.sync.dma_start(out=barr1_in[:, :],
                                                in_=onesr[:, :])
                        for pwr in hn_writes:
                            _add_dep_helper(
                                bw1.ins, pwr.ins, sync=True,
                                reason="half hn writes before pair barrier 1")
                        if not nocc:
                            barrier1_cc = nc.gpsimd.collective_compute(
                                "AllReduce", OP.add,
                                replica_groups=pair_groups,
                                ins=[barr1_in.ap().opt()],
                                outs=[barr1_out.ap().opt()])

            stat_ps = pps.tile([1, 2 * OUT_FEAT], F32, tag="stat_ps",
                               name="stat_ps")
            active = [w for w in range(nw) if caps[w].sum() > 0]
            hn_views = (hn_r[R_BASE[0]:R_END[0], :],
                        hn_r[R_BASE[1]:R_END[1], :],
                        hn_r[R_BASE[2]:R_END[2], :])

            def build_onehots(c0, ncols, mpool, tagp, m8_of):
                """Batched is_equal builds; fills {col: (tile, j)}."""
                for qi in range((ncols + OHB - 1) // OHB):
                    wq = min(OHB, ncols - qi * OHB)
                    m8 = mpool.tile([128, OHB * 128], BF16, tag="m8",
                                    name=f"m8_{tagp}_{qi}")
                    cq = c0 + qi * OHB
                    nc.vector.tensor_tensor(
                        m8[:, 0:wq * 128].rearrange("p (c f) -> p c f", c=wq),
                        io8_sb[:, 0:wq * 128].rearrange("p (c f) -> p c f", c=wq),
                        dm[:, cq:cq + wq].broadcast_to((128, wq, 128)),
                        op=OP.is_equal)
                    for j in range(wq):
                        m8_of[cq + j] = (m8, j)

            ngath = 0
            part_tiles = {}
            agg_tiles = {}

            # ---- pass 1: r0 (rows [0, CHALF)) ----
            with tc.tile_pool(name="gath0", bufs=3) as g0pool, \
                    tc.tile_pool(name="mdst0", bufs=8) as mpool, \
                    tc.tile_pool(name="aggps0", bufs=4, space="PSUM") as aps_pool:
                for gidx, ws in enumerate(groups_w):
                    gcap = sum(int(caps[w, 0]) for w in ws)
                    if gcap == 0:
                        continue
                    g0 = int(min(seg_off_map[w * 3] for w in ws
                                 if caps[w, 0] > 0))
                    gt = g0pool.tile([128, max_g0], BF16, tag="gt",
                                     name=f"gt0_{gidx}")
                    gth = nc.gpsimd.dma_gather(
                        gt[:, 0:gcap].rearrange("p (c e) -> p c e", e=OUT_FEAT),
                        hn_views[0],
                        idx_sb[:, g0 // 16:(g0 + gcap) // 16],
                        gcap, gcap, OUT_FEAT,
                        single_packet=False, queue_num=ngath % 4)
                    ngath += 1
                    dep1 = barrier1_cc if barrier1_cc is not None else bw1
                    _add_dep_helper(gth.ins, dep1.ins, sync=True,
                                    reason="r0 gather after barrier 1")
                    m8_of = {}
                    build_onehots(g0 // 128, gcap // 128, mpool,
                                  f"p0g{gidx}", m8_of)
                    for w in ws:
                        ncw = int(caps[w, 0]) // 128
                        if ncw == 0:
                            continue
                        base = int(seg_off_map[w * 3]) // 128
                        aps = aps_pool.tile([128, OUT_FEAT], F32, tag="aps",
                                           name=f"aps0_{w}")
                        for ki in range(ncw):
                            col = base + ki
                            m8, j = m8_of[col]
                            lo_off = col * 128 - g0
                            nc.tensor.matmul(
                                aps[:, :],
                                lhsT=m8[:, j * 128:(j + 1) * 128],
                                rhs=gt[:, lo_off:lo_off + OUT_FEAT],
                                start=(ki == 0), stop=(ki == ncw - 1))
                        part = wpool.tile([128, OUT_FEAT], BF16,
                                          tag=f"part{w}", name=f"part{w}")
                        if w % 2 == 0:
                            nc.vector.tensor_copy(part[:, :], aps[:, :])
                        else:
                            nc.scalar.copy(part[:, :], aps[:, :])
                        part_tiles[w] = part

            # barrier 2: the full hn table is written
            bw = nc.sync.dma_start(out=barr_in[:, :], in_=onesr[:, :])
            for wr in hn_writes:
                _add_dep_helper(bw.ins, wr.ins, sync=True,
                                reason="hn writes before pair barrier 2")
            barrier_cc = None
            if not nocc:
                barrier_cc = nc.gpsimd.collective_compute(
                    "AllReduce", OP.add, replica_groups=pair_groups,
                    ins=[barr_in.ap().opt()], outs=[barr_out.ap().opt()])

            # node-dropout masks (consumed by the finale; DVE has slack here)
            inv_keep = 1.0 / (1.0 - P_NODE)
            msk_tiles = {}
            NBF = 7
            with tc.tile_pool(name="nrl", bufs=3) as nr_pool:
                for b in range((nw + NBF - 1) // NBF):
                    ws_m = list(range(b * NBF, min((b + 1) * NBF, nw)))
                    nb = len(ws_m)
                    n0 = ws_m[0] * 128
                    nrt = nr_pool.tile([128, NBF * OUT_FEAT], F32,
                                       tag="nrt", name=f"nrt{b}")
                    nc.sync.dma_start(
                        out=nrt[:, 0:nb * OUT_FEAT].rearrange(
                            "p (t f) -> p t f", t=nb),
                        in_=noder[n0:n0 + nb * 128, :].rearrange(
                            "(t p) f -> p t f", p=128))
                    for i, w in enumerate(ws_m):
                        msk = wpool.tile([128, OUT_FEAT], BF16,
                                         tag=f"msk{w}", name=f"msk{w}")
                        nc.vector.tensor_scalar(
                            msk[:, :],
                            nrt[:, i * OUT_FEAT:(i + 1) * OUT_FEAT],
                            P_NODE, inv_keep, op0=OP.is_ge, op1=OP.mult)
                        msk_tiles[w] = msk

            # ---- pass 2: r1+r2 (rows [CHALF, NPAD)) + partial re-add ----
            with tc.tile_pool(name="gath12", bufs=3) as g12pool, \
                    tc.tile_pool(name="mdst12", bufs=8) as mpool, \
                    tc.tile_pool(name="aggps12", bufs=4, space="PSUM") as aps_pool:
                for gidx, ws in enumerate(groups_w):
                    gcap = sum(int(caps[w, 1] + caps[w, 2]) for w in ws)
                    g12_0 = None
                    gt = None
                    m8_all = {}
                    if gcap > 0:
                        g12_0 = int(min(seg_off_map[w * 3 + r] for w in ws
                                        for r in (1, 2) if caps[w, r] > 0))
                        gt = g12pool.tile([128, max_g12], BF16, tag="gt",
                                          name=f"gt12_{gidx}")
                        for r in (1, 2):
                            rcap = sum(int(caps[w, r]) for w in ws)
                            if rcap == 0:
                                continue
                            roff = int(min(seg_off_map[w * 3 + r] for w in ws
                                           if caps[w, r] > 0)) - g12_0
                            gth = nc.gpsimd.dma_gather(
                                gt[:, roff:roff + rcap].rearrange(
                                    "p (c e) -> p c e", e=OUT_FEAT),
                                hn_views[r],
                                idx_sb[:, (g12_0 + roff) // 16:
                                       (g12_0 + roff + rcap) // 16],
                                rcap, rcap, OUT_FEAT,
                                single_packet=False, queue_num=ngath % 4)
                            ngath += 1
                            dep2 = (barrier_cc if barrier_cc is not None
                                    else bw)
                            _add_dep_helper(gth.ins, dep2.ins, sync=True,
                                            reason="r12 gather after barrier 2")
                            build_onehots((g12_0 + roff) // 128, rcap // 128,
                                          mpool, f"p{r}g{gidx}", m8_all)
                    for w in ws:
                        chunk_cols = []
                        for r in (1, 2):
                            cap = int(caps[w, r])
                            for k in range(cap // 128):
                                chunk_cols.append(
                                    int(seg_off_map[w * 3 + r]) // 128 + k)
                        part = part_tiles.get(w)
                        if not chunk_cols and part is None:
                            continue
                        aps = aps_pool.tile([128, OUT_FEAT], F32, tag="aps",
                                            name=f"aps12_{w}")
                        nmm = len(chunk_cols) + (1 if part is not None else 0)
                        mi = 0
                        for col in chunk_cols:
                            m8, j = m8_all[col]
                            lo_off = col * 128 - g12_0
                            nc.tensor.matmul(
                                aps[:, :],
                                lhsT=m8[:, j * 128:(j + 1) * 128],
                                rhs=gt[:, lo_off:lo_off + OUT_FEAT],
                                start=(mi == 0), stop=(mi == nmm - 1))
                            mi += 1
                        if part is not None:
                            nc.tensor.matmul(
                                aps[:, :], lhsT=idn16[:, :], rhs=part[:, :],
                                start=(mi == 0), stop=True)
                        # agg|sq packed in one tile for a single stats matmul
                        asq = wpool.tile([128, 2 * OUT_FEAT], BF16,
                                         tag=f"asq{w}", name=f"asq{w}")
                        nc.scalar.activation(
                            asq[:, 0:OUT_FEAT], aps[:, 0:OUT_FEAT],
                            AF.Copy, scale=d3T[:, w:w + 1])
                        nc.scalar.square(asq[:, OUT_FEAT:2 * OUT_FEAT],
                                         asq[:, 0:OUT_FEAT])
                        nc.tensor.matmul(stat_ps[0:1, :],
                                         lhsT=ones16[:, :], rhs=asq[:, :],
                                         start=(w == active[0]),
                                         stop=(w == active[-1]))
                        agg_tiles[w] = asq

            # ---------- phase E: BN stats + finale ----------
            stat_sb = wpool.tile([1, 2 * OUT_FEAT], F32, tag="stat_sb",
                                 name="stat_sb")
            nc.vector.tensor_copy(stat_sb[:, :], stat_ps[:, :])
            nc.sync.dma_start(out=statb_in[:, :], in_=stat_sb[:, :])
            if nocc:
                nc.sync.dma_start(out=statb_out[:, :], in_=statb_in[:, :])
            else:
                nc.gpsimd.collective_compute(
                    "AllReduce", OP.add, replica_groups=groups,
                    ins=[statb_in.ap().opt()], outs=[statb_out.ap().opt()])
            stat2 = wpool.tile([1, 2 * OUT_FEAT], F32, tag="stat2",
                               name="stat2")
            nc.sync.dma_start(out=stat2[:, :], in_=statb_out[:, :])

            inv_n = 1.0 / float(n_nodes)
            mn = wpool.tile([1, OUT_FEAT], F32, tag="mn", name="mn")
            nc.vector.tensor_scalar(mn[:, :], stat2[0:1, 0:OUT_FEAT], inv_n,
                                    None, op0=OP.mult)
            ex2 = wpool.tile([1, OUT_FEAT], F32, tag="ex2", name="ex2")
            nc.vector.tensor_scalar(ex2[:, :], stat2[0:1, OUT_FEAT:2 * OUT_FEAT],
                                    inv_n, None, op0=OP.mult)
            var = wpool.tile([1, OUT_FEAT], F32, tag="var", name="var")
            nc.vector.tensor_mul(var[:, :], mn[:, :], mn[:, :])
            nc.vector.tensor_sub(var[:, :], ex2[:, :], var[:, :])
            nc.vector.tensor_scalar_add(var[:, :], var[:, :], BN_EPS)
            sd = wpool.tile([1, OUT_FEAT], F32, tag="sd", name="sd")
            nc.scalar.sqrt(sd[:, :], var[:, :])
            istd = wpool.tile([1, OUT_FEAT], F32, tag="istd", name="istd")
            nc.vector.reciprocal(istd[:, :], sd[:, :])
            st_row = wpool.tile([1, 2 * OUT_FEAT], F32, tag="st_row",
                                name="st_row")
            # s = gamma * istd ; t = beta - mean * s
            nc.vector.tensor_mul(st_row[0:1, 0:OUT_FEAT], gam_sb[:, :],
                                 istd[:, :])
            tmp_t = wpool.tile([1, OUT_FEAT], F32, tag="tmp_t", name="tmp_t")
            nc.vector.tensor_mul(tmp_t[:, :], mn[:, :],
                                 st_row[0:1, 0:OUT_FEAT])
            nc.vector.tensor_sub(st_row[0:1, OUT_FEAT:2 * OUT_FEAT],
                                 bet_sb[:, :], tmp_t[:, :])
            with tc.tile_pool(name="bps", bufs=1, space="PSUM") as bps_pool:
                bps = bps_pool.tile([128, 2 * OUT_FEAT], F32, tag="bps",
                                    name="bps")
                nc.tensor.matmul(bps[:, :], lhsT=onesr[:, :], rhs=st_row[:, :],
                                 start=True, stop=True)
                st_bc = wpool.tile([128, 2 * OUT_FEAT], BF16, tag="st_bc",
                                   name="st_bc")
                nc.vector.tensor_copy(st_bc[:, :], bps[:, :])

            with tc.tile_pool(name="fin", bufs=3) as fpool:
                for b in range((nw + NBF - 1) // NBF):
                    ws = list(range(b * NBF, min((b + 1) * NBF, nw)))
                    nb = len(ws)
                    n0 = ws[0] * 128
                    ot = fpool.tile([128, NBF * OUT_FEAT], F32, tag="ot",
                                    name=f"ot{b}")
                    for i, w in enumerate(ws):
                        agg = agg_tiles.get(w)
                        o_sl = ot[:, i * OUT_FEAT:(i + 1) * OUT_FEAT]
                        y = fpool.tile([128, OUT_FEAT], BF16, tag="y",
                                       name=f"y{w}")
                        if agg is None:
                            nc.vector.tensor_copy(
                                y[:, :], st_bc[:, OUT_FEAT:2 * OUT_FEAT])
                        else:
                            nc.vector.tensor_mul(y[:, :], agg[:, 0:OUT_FEAT],
                                                 st_bc[:, 0:OUT_FEAT])
                            nc.vector.tensor_add(
                                y[:, :], y[:, :],
                                st_bc[:, OUT_FEAT:2 * OUT_FEAT])
                        nc.gpsimd.tensor_mul(o_sl, y[:, :], msk_tiles[w][:, :])
                    nc.scalar.dma_start(
                        out=out[n0:n0 + nb * 128, :].rearrange(
                            "(t p) f -> p t f", p=128),
                        in_=ot[:, 0:nb * OUT_FEAT].rearrange(
                            "p (t f) -> p t f", t=nb))

    nc.compile()
    return nc


_CACHE = {}


def _get_program(inputs):
    key = tuple(np.asarray(inputs["src"])[:8].tolist()) + (
        len(np.asarray(inputs["src"])),)
    if key not in _CACHE:
        sh, in_maps = prep_inputs(
            inputs["features"], inputs["W"], inputs["gamma"], inputs["beta"],
            inputs["src"], inputs["dst"], inputs["edge_rand"],
            inputs["node_rand"])
        nc = build_program(sh)
        _CACHE[key] = (sh, nc)
    else:
        sh, nc = _CACHE[key]
        _, in_maps = prep_inputs(
            inputs["features"], inputs["W"], inputs["gamma"], inputs["beta"],
            inputs["src"], inputs["dst"], inputs["edge_rand"],
            inputs["node_rand"])
    return sh, _CACHE[key][1], in_maps


def kernel(**inputs):
    sh, nc, in_maps = _get_program(inputs)
    res = run_bass_kernel_spmd(nc, in_maps, core_ids=list(range(CORES)))
    npc = sh["npc"]
    full = np.concatenate([res.results[c]["out"] for c in range(CORES)],
                          axis=0)
    return np.ascontiguousarray(full[:sh["n_nodes"]]).astype(np.float32)
